# revision 17
# baseline (speedup 1.0000x reference)
"""Causal multi-head attention on 8 TRN2 NeuronCores.

Sharding: tensor-parallel over heads (2 heads/core) for QKV projection and
attention; AllToAll redistributes attention outputs so each core owns L/8
rows of every batch for the output projection. Full inputs in, full output
out; all FLOPs on device.

Compute dtype: bf16 operands with fp32 PSUM accumulation (TensorE runs at
full rate and weight loads use the fast-weight-load path; fp32/fp32r
stationaries are self-loading and serialize ~220ns per matmul, which kept
the PE array duty cycle low enough that the HAM clock gate pinned the
whole attention phase at 1.2 GHz). Softmax denominators, reciprocals and
normalization stay fp32.

Per-core pipeline (per batch):
  1. Load X rows (bf16), PE-transpose to Xt [k, l], QKV projection with
     stationary W tiles -> QKVt [384c, L] with d_attn on partitions (bias
     added in fp32 on DVE during the PSUM eviction).
  2. PE-transpose V-pair columns to natural layout V' [l, 65] with an
     appended ones column (softmax denominator comes out of the attention
     matmul for free).
  3. Flash-style causal attention per head: S.T = K-stationary.T @ Q,
     exp fused on ScalarE (scale=1/sqrt(d), no max subtraction -- scores
     are bounded for this problem), diagonal tiles restricted to their
     causal column range and masked, O'.T accumulated over key tiles in
     fp32 PSUM; normalization deferred: numerators evicted unnormalized,
     denominator reciprocal broadcast via DRAM bounce, one DVE multiply.
  4. AllToAll (per batch): own-128-channel x 256-row blocks redistributed
     so each core gets all 1024 channels for its own L/8 rows.
  5. Output projection (stationary gathered O.T tiles, moving W_out) plus
     broadcast fp32 bias, DMA to the per-core output row slice.
  The output projection of batch b-1 is emitted after stage 2 of batch b
  so the in-order PE stream never waits on an AllToAll.
"""
from contextlib import ExitStack

import ml_dtypes
import numpy as np

import concourse.bass as bass
import concourse.tile as tile
from concourse import bacc, mybir
from concourse.bass_utils import run_bass_kernel_spmd

N_CORES = 8
D_MODEL = 1024
N_HEADS = 16
D_ATTN = 64
SEQ_LEN = 2048
BATCH = 4

F32 = mybir.dt.float32
BF16 = mybir.dt.bfloat16
ActF = mybir.ActivationFunctionType
BF16_NP = np.dtype(ml_dtypes.bfloat16)


def build_program(B=BATCH, L=SEQ_LEN):
    """Build the SPMD Bass program. Parametric over batch/seq for sim tests.
    Requires L % 1024 == 0. Full problem: B=4, L=2048."""
    assert L % 1024 == 0
    D = D_MODEL
    CQ = 1024                 # l_q chunk for attention
    NCH = L // CQ             # number of l_q chunks (4 at L=2048)
    NJT = L // 128            # number of l_k tiles (16)
    LO = L // N_CORES         # own rows per batch per core (256)
    KT = D // 128             # contraction k-tiles (8)
    NDT = CQ // 128           # l_k tiles per diagonal chunk (8)

    nc = bacc.Bacc("TRN2", target_bir_lowering=False, debug=False,
                   num_devices=N_CORES)

    X = nc.dram_tensor("X", [B, L, D], BF16, kind="ExternalInput").ap()
    WQKV = nc.dram_tensor("WQKV", [D, 384], BF16, kind="ExternalInput").ap()
    BQKV = nc.dram_tensor("BQKV", [3, 128, 1], F32, kind="ExternalInput").ap()
    WOUT = nc.dram_tensor("WOUT", [D, D], BF16, kind="ExternalInput").ap()
    BOUT = nc.dram_tensor("BOUT", [D], F32, kind="ExternalInput").ap()
    IDENT = nc.dram_tensor("IDENT", [128, 128], BF16, kind="ExternalInput").ap()
    ONES = nc.dram_tensor("ONES", [128, NJT, 2, 1], BF16,
                          kind="ExternalInput").ap()
    MASKS = nc.dram_tensor("MASKS", [128, 128], BF16,
                           kind="ExternalInput").ap()
    OUT = nc.dram_tensor("OUT", [B, LO, D], F32, kind="ExternalOutput").ap()

    a2a_in = [nc.dram_tensor(f"a2a_in{b}", [N_CORES, 128, LO], BF16).ap()
              for b in range(B)]
    a2a_out = [nc.dram_tensor(f"a2a_out{b}", [N_CORES, 128, LO], BF16).ap()
               for b in range(B)]
    dn_bounce = [nc.dram_tensor(f"dn{b}", [2, 1, L], F32).ap()
                 for b in range(B)]

    with tile.TileContext(nc) as tc, ExitStack() as ctx:
        const = ctx.enter_context(tc.tile_pool(name="const", bufs=1))
        qkvt_pool = ctx.enter_context(tc.tile_pool(name="qkvt", bufs=2))
        xt_pool = ctx.enter_context(tc.tile_pool(name="xt", bufs=3))
        vp_pool = ctx.enter_context(tc.tile_pool(name="vp", bufs=2))
        p_pool = ctx.enter_context(tc.tile_pool(name="p", bufs=4))
        ot_pool = ctx.enter_context(tc.tile_pool(name="ot", bufs=2))
        on_pool = ctx.enter_context(tc.tile_pool(name="on", bufs=2))
        rb_pool = ctx.enter_context(tc.tile_pool(name="rb", bufs=2))
        otg_pool = ctx.enter_context(tc.tile_pool(name="otg", bufs=2))
        osb_pool = ctx.enter_context(tc.tile_pool(name="osb", bufs=2))

        ps_acc = ctx.enter_context(
            tc.tile_pool(name="ps_acc", bufs=1, space="PSUM"))
        ps_s = ctx.enter_context(
            tc.tile_pool(name="ps_s", bufs=2, space="PSUM"))
        ps_mm = ctx.enter_context(
            tc.tile_pool(name="ps_mm", bufs=2, space="PSUM"))

        # ---- constants / weights ----
        ident = const.tile([128, 128], BF16, tag="ident")
        nc.sync.dma_start(out=ident, in_=IDENT)
        bq_sb = const.tile([128, 3], F32, tag="bq")
        for cc in range(3):
            nc.gpsimd.dma_start(out=bq_sb[:, cc:cc + 1], in_=BQKV[cc])
        wsb = const.tile([128, KT, 384], BF16, tag="wsb")
        for t in range(KT):
            nc.gpsimd.dma_start(out=wsb[:, t, :],
                                in_=WQKV[128 * t:128 * (t + 1), :])
        masks = const.tile([128, 128], BF16, tag="masks")
        nc.gpsimd.dma_start(out=masks, in_=MASKS)
        # W_out / b_out are not needed until the first out-projection;
        # load on the gpsimd queue so startup X loads are not delayed.
        wout_sb = const.tile([128, KT, D], BF16, tag="wout")
        for t in range(KT):
            nc.gpsimd.dma_start(out=wout_sb[:, t, :],
                                in_=WOUT[128 * t:128 * (t + 1), :])
        bout_bc = const.tile([128, D], F32, tag="bout")
        nc.gpsimd.dma_start(
            out=bout_bc,
            in_=bass.AP(tensor=BOUT.tensor, offset=0, ap=[[0, 128], [1, D]]))

        def stage5(b):
            # ---- stage 5: output projection of batch b ----
            otg = otg_pool.tile([128, KT, LO], BF16, tag="otg")
            for si in range(N_CORES):
                nc.gpsimd.dma_start(out=otg[:, si, :], in_=a2a_out[b][si])
            for lt in range(LO // 128):
                for nk in range(D // 512):
                    po = ps_mm.tile([128, 512], F32, tag="mm")
                    for ct in range(KT):
                        nc.tensor.matmul(
                            po, otg[:, ct, 128 * lt:128 * (lt + 1)],
                            wout_sb[:, ct, 512 * nk:512 * (nk + 1)],
                            start=(ct == 0), stop=(ct == KT - 1))
                    osb = osb_pool.tile([128, 512], F32, tag="osb")
                    nc.vector.tensor_add(
                        osb, po, bout_bc[:, 512 * nk:512 * (nk + 1)])
                    nc.gpsimd.dma_start(
                        out=OUT[b, 128 * lt:128 * (lt + 1),
                                512 * nk:512 * (nk + 1)],
                        in_=osb)

        for b in range(B):
            # ---- stage 1: transpose X + QKV projection ----
            qkvt = qkvt_pool.tile([128, 3, L], BF16, tag="qkvt")
            for lc in range(L // 256):
                xt = xt_pool.tile([128, KT, 256], BF16, tag="xt")
                r0 = 256 * lc
                for t in range(KT):
                    nc.sync.dma_start(
                        out=xt[:, t, :],
                        in_=X[b, r0:r0 + 256, 128 * t:128 * (t + 1)],
                        transpose=True)
                for cc in range(3):
                    pq = ps_mm.tile([128, 512], F32, tag="mm")
                    pqv = pq[:, 0:256]
                    for t in range(KT):
                        nc.tensor.matmul(
                            pqv, wsb[:, t, 128 * cc:128 * (cc + 1)],
                            xt[:, t, :], start=(t == 0), stop=(t == KT - 1))
                    nc.vector.tensor_scalar_add(
                        qkvt[:, cc, 256 * lc:256 * (lc + 1)], pqv,
                        bq_sb[:, cc:cc + 1])

            # ---- stage 2: V' assembly ----
            vp = vp_pool.tile([128, NJT, 2, 65], BF16, tag="vp")
            nc.sync.dma_start(out=vp[:, :, :, 64:65], in_=ONES)
            for jt in range(NJT):
                pv = ps_mm.tile([128, 512], BF16, tag="mm")
                pvv = pv[:, 0:128]
                nc.tensor.transpose(
                    pvv, qkvt[:, 2, 128 * jt:128 * (jt + 1)], ident)
                pv3 = bass.AP(tensor=pvv.tensor, offset=pvv.offset,
                              ap=[list(pvv.ap[0]), [64, 2], [1, 64]])
                nc.any.tensor_copy(vp[:, jt, :, 0:64], pv3)

            # ---- stage 3: attention per head ----
            ots = []
            for hp in range(2):
                hs = slice(64 * hp, 64 * (hp + 1))
                qh = qkvt[hs, 0, :]
                kh = qkvt[hs, 1, :]
                onum = on_pool.tile([65, L], F32, tag="onum")
                ot = ot_pool.tile([64, L], BF16, tag=f"ot{hp}")
                for q in range(NCH):
                    acc = ps_acc.tile([65, CQ], F32, tag="acc")
                    last_jt = (CQ * (q + 1)) // 128 - 1
                    # last jt writing each 512-wide PSUM bank piece
                    last_for = [min(last_jt, NDT * q + bk * 4 + 3)
                                for bk in range(CQ // 512)]
                    for jt in range(last_jt + 1):
                        diag = jt // NDT == q
                        o = jt - NDT * q if diag else 0
                        c0 = 128 * o
                        # bank-aligned output pieces (<=512 f32 per bank)
                        pcs = []
                        lo = c0
                        while lo < CQ:
                            hi = min(CQ, (lo // 512 + 1) * 512)
                            pcs.append((lo, hi))
                            lo = hi
                        sp = ps_s.tile([128, CQ], F32, tag="s")
                        for (lo, hi) in pcs:
                            nc.tensor.matmul(
                                sp[:, lo:hi], kh[:, 128 * jt:128 * (jt + 1)],
                                qh[:, CQ * q + lo:CQ * q + hi],
                                start=True, stop=True)
                        psb = p_pool.tile([128, CQ], BF16, tag="p")
                        nc.scalar.activation(out=psb[:, c0:], in_=sp[:, c0:],
                                             func=ActF.Exp, scale=0.125)
                        if diag:
                            nc.vector.tensor_mul(
                                psb[:, c0:c0 + 128], psb[:, c0:c0 + 128],
                                masks)
                        for (lo, hi) in pcs:
                            bk = lo // 512
                            stop = jt == last_for[bk]
                            nc.tensor.matmul(
                                acc[:, lo:hi], vp[:, jt, hp, :],
                                psb[:, lo:hi],
                                start=(jt == 0), stop=stop)
                            if stop:
                                p0 = 512 * bk
                                nc.vector.tensor_copy(
                                    onum[:, CQ * q + p0:CQ * q + p0 + 512],
                                    acc[0:65, p0:p0 + 512])
                    nc.sync.dma_start(
                        out=dn_bounce[b][hp][:, CQ * q:CQ * (q + 1)],
                        in_=onum[64:65, CQ * q:CQ * (q + 1)])
                # broadcast raw denominators back, reciprocal, normalize
                rb = rb_pool.tile([64, L], F32, tag="rb")
                nc.sync.dma_start(
                    out=rb,
                    in_=bass.AP(tensor=dn_bounce[b].tensor, offset=hp * L,
                                ap=[[0, 64], [1, L]]))
                nc.vector.reciprocal(out=rb, in_=rb)
                nc.vector.tensor_mul(ot, onum[0:64, :], rb)
                ots.append(ot)
                if hp == 1 and b > 0:
                    stage5(b - 1)

            # ---- stage 4: AllToAll ----
            for d in range(N_CORES):
                for hp in range(2):
                    nc.sync.dma_start(
                        out=a2a_in[b][d, 64 * hp:64 * (hp + 1), :],
                        in_=ots[hp][:, LO * d:LO * (d + 1)])
            nc.gpsimd.collective_compute(
                "AllToAll", mybir.AluOpType.bypass,
                replica_groups=[list(range(N_CORES))],
                ins=[a2a_in[b]], outs=[a2a_out[b]])

        stage5(B - 1)

    nc.compile()
    return nc


def make_inputs(X, W_qkv, b_qkv, W_out, b_out, B=BATCH, L=SEQ_LEN):
    """Build per-core input maps from full inputs."""
    NJT = L // 128
    Wr = np.ascontiguousarray(W_qkv).reshape(D_MODEL, N_HEADS, 3, D_ATTN)
    br = np.ascontiguousarray(b_qkv).reshape(N_HEADS, 3, D_ATTN)
    ident = np.eye(128, dtype=np.float32).astype(BF16_NP)
    ones = np.ones((128, NJT, 2, 1), dtype=np.float32).astype(BF16_NP)
    p = np.arange(128)[:, None]
    n = np.arange(128)[None, :]
    masks = (p <= n).astype(np.float32).astype(BF16_NP)
    Xb = np.ascontiguousarray(X, dtype=np.float32).astype(BF16_NP)
    Wo = np.ascontiguousarray(W_out, dtype=np.float32).astype(BF16_NP)
    in_maps = []
    for c in range(N_CORES):
        ha, hb = 2 * c, 2 * c + 1
        # column order per ctile: [Qa|Qb], [Ka|Kb], [Va|Vb]
        wp = np.concatenate(
            [Wr[:, ha, 0], Wr[:, hb, 0],
             Wr[:, ha, 1], Wr[:, hb, 1],
             Wr[:, ha, 2], Wr[:, hb, 2]], axis=1)
        bp = np.concatenate(
            [br[ha, 0], br[hb, 0], br[ha, 1], br[hb, 1],
             br[ha, 2], br[hb, 2]])
        in_maps.append({
            "X": Xb,
            "WQKV": np.ascontiguousarray(wp, dtype=np.float32)
                      .astype(BF16_NP),
            "BQKV": np.ascontiguousarray(bp, dtype=np.float32)
                      .reshape(3, 128, 1),
            "WOUT": Wo,
            "BOUT": np.ascontiguousarray(b_out, dtype=np.float32),
            "IDENT": ident,
            "ONES": ones,
            "MASKS": masks,
        })
    return in_maps


def assemble_output(results, B=BATCH, L=SEQ_LEN):
    LO = L // N_CORES
    out = np.empty((B, L, D_MODEL), dtype=np.float32)
    for c in range(N_CORES):
        o = results[c]["OUT"]
        for b in range(B):
            out[b, LO * c:LO * (c + 1), :] = o[b]
    return out


_CACHED_NC = None


def kernel(X, W_qkv, b_qkv, W_out, b_out):
    global _CACHED_NC
    X = np.asarray(X, dtype=np.float32)
    if _CACHED_NC is None:
        _CACHED_NC = build_program(BATCH, SEQ_LEN)
    in_maps = make_inputs(X, np.asarray(W_qkv), np.asarray(b_qkv),
                          np.asarray(W_out), np.asarray(b_out))
    res = run_bass_kernel_spmd(_CACHED_NC, in_maps, list(range(N_CORES)))
    return assemble_output(res.results)


if __name__ == "__main__":
    nc = build_program(1, 2048)
    print("built + compiled ok")
